# revision 12
# baseline (speedup 1.0000x reference)
"""Deformable conv block (3x3 offset conv -> 3x3 deformable group conv), 8x trn2.

Sharding: data-parallel over (batch=2) x (H quarters=4) -> 8 cores; each core
gets a zero-padded slab (3-row/3-col halo) so sampling's zero-outside-image
semantics fall out of the padding.

v2 pipeline: the per-pixel bilinear combination is done with a few wide fp16
DVE ops per 128-px tile instead of 189 scalar_tensor_tensor ops:
  - T images stored o-major [px, o(72) x slot(48)] fp16; slot = ky*16 + i,
    i indexes the 15 (kx, v) pairs sorted by column shift; slot 15 per ky is a
    zero pad (weights zeroed) for 4B-aligned access patterns.
  - q coefficients per pixel: 240 = ky(3) x u(5) x 16 fp16 values; pad rows
    forced to zero via an out-of-range tent bias. Staged through DRAM to keep
    SBUF pressure down.
  - per (ky, u): ONE tensor_tensor mult [px, o:72 x slot:16] with q read via a
    stride-0 outer-dup access pattern (2x_1p: all operands packed fp16).
  - reduction: fp16 add tree over the 240-term axis, final 5-term
    tensor_reduce into f32, DMA out.
Window terms cover the full (u,v) in {-2..2}^2 grid per tap: exact for
|offset| < 2 (corner combos where both |dy|>1 and |dx|>1 never co-occur, so
their q is zero anyway).
"""

import numpy as np
from contextlib import ExitStack

import concourse.bass as bass
import concourse.tile as tile
from concourse import bacc, mybir
from concourse import bass_utils
from concourse.bass import AP

# Problem constants
B, C, O, H, W = 2, 72, 72, 180, 320
NK = 9                # deform taps
OC = 18               # offset channels
PADC = 3
WP = W + 2 * PADC     # 326
NQ = 4
RS = H // NQ          # 45
HALO = 3
RSP = RS + 2 * HALO   # 51
NPIX_I = RSP * WP
FROWS = RS + 2        # feat slab rows (conv needs +-1)
NPIX_F = FROWS * WP
N_CORES = 8

F32 = mybir.dt.float32
F16 = mybir.dt.float16

# --- slot / coefficient layout -------------------------------------------
# Per tap-row ky, the 15 (kx, v) pairs sorted by column shift s = kx-1 + v-2.
SLOT15 = sorted(((kx - 1 + v - 2, kx, v) for kx in range(3) for v in range(5)))
NSPK = 16             # slots per ky: 15 real + 1 pad
NSLOT = 3 * NSPK      # 48
NTPK = 5 * NSPK       # 80 terms per ky (u x 16)
NQROW = 3 * NTPK      # 240
NQROW2 = 2 * NQROW    # pair-duplicated for DVE packed-pair reads
CGRPS = [(0, 128), (128, NQROW - 128)]

# shift per slot (pads inherit previous slot's shift; weights zero there)
SLOT_SHIFT = []
for _ky in range(3):
    for (_s, _kx, _v) in SLOT15:
        SLOT_SHIFT.append(_s)
    SLOT_SHIFT.append(SLOT_SHIFT[-1])

SPB = 7   # slots per PSUM bank (7*72=504 <= 512)

T_RUNS = []   # (shift, slot_lo, slot_hi) contiguous, same shift+bank
_j = 0
while _j < NSLOT:
    _s = SLOT_SHIFT[_j]
    _hi = _j
    while _hi < NSLOT and SLOT_SHIFT[_hi] == _s and _hi // SPB == _j // SPB:
        _hi += 1
    T_RUNS.append((_s, _j, _hi))
    _j = _hi
T_RUNS_EMIT = sorted(T_RUNS, key=lambda r: (r[0], r[1]))

COL_TILES = [(PADC, 128), (PADC + 128, 128), (PADC + 256, 64)]
NUNIT = RS * 3


def _psum_col(slot):
    return (slot // SPB) * 512 + (slot % SPB) * O


def build_module():
    nc = bacc.Bacc("TRN2", target_bir_lowering=False, debug=False,
                   num_devices=N_CORES)

    img_d = nc.dram_tensor("img", [C, NPIX_I], F16, kind="ExternalInput")
    feat_d = nc.dram_tensor("feat", [C, NPIX_F], F16, kind="ExternalInput")
    wts_d = nc.dram_tensor("wts", [C, NSLOT * O], F16, kind="ExternalInput")
    offw_d = nc.dram_tensor("offw", [C, 9 * OC], F16, kind="ExternalInput")
    offb_d = nc.dram_tensor("offb", [OC, 1], F32, kind="ExternalInput")
    repy_d = nc.dram_tensor("repy", [OC, NQROW], F16, kind="ExternalInput")
    repx_d = nc.dram_tensor("repx", [OC, NQROW], F16, kind="ExternalInput")
    biasu_d = nc.dram_tensor("biasu", [NQROW, 1], F32, kind="ExternalInput")
    biasv_d = nc.dram_tensor("biasv", [NQROW, 1], F32, kind="ExternalInput")
    ident_d = nc.dram_tensor("ident", [128, 128], F32, kind="ExternalInput")
    out_d = nc.dram_tensor("out", [RS * W, O], F32, kind="ExternalOutput")
    q_d = nc.dram_tensor("qsc", [NUNIT * 128, NQROW2], F16, kind="Internal")

    with tile.TileContext(nc) as tc, ExitStack() as ctx:
        const = ctx.enter_context(tc.tile_pool(name="const", bufs=1))
        big = ctx.enter_context(tc.tile_pool(name="big", bufs=1))

        wts = const.tile([C, NSLOT * O], F16)
        nc.sync.dma_start(wts[:], wts_d[:])
        offw = const.tile([C, 9 * OC], F16)
        nc.sync.dma_start(offw[:], offw_d[:])
        offb = const.tile([OC, 1], F32)
        nc.sync.dma_start(offb[:], offb_d[:])
        repy = const.tile([OC, NQROW], F16)
        nc.sync.dma_start(repy[:], repy_d[:])
        repx = const.tile([OC, NQROW], F16)
        nc.sync.dma_start(repx[:], repx_d[:])
        biasu = {}
        biasv = {}
        for g0, gn in CGRPS:
            bu = const.tile([gn, 1], F32, tag=f"biasu{g0}")
            nc.sync.dma_start(bu[:], biasu_d[g0:g0 + gn, :])
            biasu[g0] = bu
            bv = const.tile([gn, 1], F32, tag=f"biasv{g0}")
            nc.sync.dma_start(bv[:], biasv_d[g0:g0 + gn, :])
            biasv[g0] = bv
        ident = const.tile([128, 128], F32)
        nc.sync.dma_start(ident[:], ident_d[:])

        imgh = big.tile([C, NPIX_I], F16)
        nc.sync.dma_start(imgh[:], img_d[:])

        # ---------------- phase BC: offsets -> per-pixel q rows ----------
        with tc.tile_pool(name="featp", bufs=1) as featp, \
             tc.tile_pool(name="ps_off", bufs=2, space="PSUM") as ps_off, \
             tc.tile_pool(name="ps_rep", bufs=2, space="PSUM") as ps_rep, \
             tc.tile_pool(name="ps_tr", bufs=2, space="PSUM") as ps_tr, \
             tc.tile_pool(name="sc", bufs=3) as sc:
            feat = featp.tile([C, NPIX_F], F16)
            nc.sync.dma_start(feat[:], feat_d[:])

            CW = WP - 2  # conv output cols [1, 325) of the padded row
            for r in range(RS):
                fbase = (r + 1) * WP + 1
                po = ps_off.tile([OC, CW], F32, tag="po")
                for t in range(9):
                    d = (t // 3 - 1) * WP + (t % 3 - 1)
                    nc.tensor.matmul(
                        po[:, :],
                        offw[:, t * OC:(t + 1) * OC],
                        feat[:, fbase + d: fbase + d + CW],
                        start=(t == 0), stop=(t == 8))
                offs = sc.tile([OC, CW], F16, tag="offs")
                nc.vector.tensor_scalar(
                    out=offs[:], in0=po[:, :], scalar1=offb[:], scalar2=None,
                    op0=mybir.AluOpType.add)

                qg = {}
                for g0, gn in CGRPS:
                    ty = sc.tile([gn, CW], F32, tag=f"ty{g0}")
                    tx = sc.tile([gn, CW], F32, tag=f"tx{g0}")
                    for (rep, bia, dst) in ((repy, biasu[g0], ty),
                                            (repx, biasv[g0], tx)):
                        pr = ps_rep.tile([128, CW], F32, tag="pr")
                        nc.tensor.matmul(
                            pr[:gn, :],
                            rep[:, g0:g0 + gn],
                            offs[:],
                            start=True, stop=True)
                        nc.scalar.activation(
                            dst[:, :], pr[:gn, :],
                            mybir.ActivationFunctionType.Abs,
                            bias=bia[:], scale=1.0)
                        nc.scalar.activation(
                            dst[:, :], dst[:, :],
                            mybir.ActivationFunctionType.Relu,
                            bias=1.0, scale=-1.0)
                    q = sc.tile([gn, CW], F32, tag=f"q{g0}")
                    nc.vector.tensor_tensor(out=q[:], in0=ty[:], in1=tx[:],
                                            op=mybir.AluOpType.mult)
                    qg[g0] = q

                for ct, (c0, tw) in enumerate(COL_TILES):
                    unit = r * 3 + ct
                    qstage = sc.tile([128, NQROW2], F16, tag="qstage")
                    qsa = qstage[:tw, :]
                    for g0, gn in CGRPS:
                        pt = ps_tr.tile([128, 128], F32, tag="pt")
                        nc.tensor.transpose(
                            pt[:tw, :gn], qg[g0][:, c0 - 1:c0 - 1 + tw],
                            ident[:gn, :gn])
                        # pair-duplicate: q2[2j] = q2[2j+1] = q[j]
                        for par in range(2):
                            dst = AP(qsa.tensor,
                                     qsa.offset + 2 * g0 + par,
                                     [qsa.ap[0], [2, gn]])
                            nc.scalar.copy(dst, pt[:tw, :gn])
                    nc.sync.dma_start(
                        q_d[unit * 128: unit * 128 + tw, :], qstage[:tw, :])

        # ---------------- phase DE: T images + wide combine --------------
        with tc.tile_pool(name="ps_T", bufs=1, space="PSUM") as ps_T, \
             tc.tile_pool(name="tpool", bufs=9) as tpool, \
             tc.tile_pool(name="qpool", bufs=3) as qpool, \
             tc.tile_pool(name="apool", bufs=1) as apool, \
             tc.tile_pool(name="rpool", bufs=1) as rpool:

            for ct, (c0, tw) in enumerate(COL_TILES):
                t_tiles = {}

                def build_T(rp, c0=c0, tw=tw, t_tiles=t_tiles):
                    base = (rp + HALO) * WP + c0
                    pT = ps_T.tile([128, 7 * 512], F32, tag="pT")
                    for (s, jlo, jhi) in T_RUNS_EMIT:
                        nc.tensor.matmul(
                            pT[:tw, _psum_col(jlo):
                                    _psum_col(jlo) + (jhi - jlo) * O],
                            imgh[:, base + s: base + s + tw],
                            wts[:, jlo * O: jhi * O],
                            start=True, stop=True)
                    # drain each bank -> slot-major fp16 [slot(48) x o(72)]
                    tsb = tpool.tile([128, O * NSLOT], F16, tag="tsb")
                    for bk in range(7):
                        lo = bk * SPB
                        ns = min(SPB * (bk + 1), NSLOT) - lo
                        nc.scalar.copy(
                            tsb[:tw, lo * O: lo * O + ns * O],
                            pT[:tw, bk * 512: bk * 512 + ns * O])
                    t_tiles[rp] = tsb

                for rp in range(-3, 3):
                    build_T(rp)
                for r in range(RS):
                    build_T(r + 3)
                    unit = r * 3 + ct
                    qrow = qpool.tile([128, NQROW2], F16, tag="qrow")
                    nc.sync.dma_start(
                        qrow[:tw, :], q_d[unit * 128: unit * 128 + tw, :])
                    qa = qrow[:tw, :]

                    prods = []
                    for ky in range(3):
                        prod = apool.tile([128, O * NTPK], F16,
                                          tag=f"prod{ky}")
                        pa = prod[:tw, :]
                        for u in range(5):
                            tsb = t_tiles[r + (ky - 1) + (u - 2)]
                            ta = tsb[:tw, :]
                            # stream (slot, o); q via pair-duplicated reads
                            in0 = AP(ta.tensor, ta.offset + ky * NSPK * O,
                                     [ta.ap[0], [O, NSPK], [2, O // 2],
                                      [1, 2]])
                            in1 = AP(qa.tensor,
                                     qa.offset + 2 * (ky * NTPK + u * NSPK),
                                     [qa.ap[0], [2, NSPK], [0, O // 2],
                                      [1, 2]])
                            outp = AP(pa.tensor, pa.offset + u * NSPK * O,
                                      [pa.ap[0], [O, NSPK], [2, O // 2],
                                       [1, 2]])
                            nc.vector.tensor_tensor(
                                out=outp, in0=in0, in1=in1,
                                op=mybir.AluOpType.mult)
                        prods.append(prod)

                    # flat halving tree over the term axis
                    def fold(src_tile, n_el, dst_tile, base=0, eng=None):
                        half = n_el // 2
                        (eng or nc.vector).tensor_tensor(
                            out=dst_tile[:tw, 0:half],
                            in0=src_tile[:tw, base:base + half],
                            in1=src_tile[:tw, base + half:base + n_el],
                            op=mybir.AluOpType.add)

                    hs = []
                    for ky in range(3):
                        h = rpool.tile([128, O * 40], F16, tag=f"h{ky}")
                        # band 2's fold runs on the idle GPSIMD engine, in
                        # parallel with the DVE folds of bands 0/1
                        fold(prods[ky], NTPK * O, h,
                             eng=nc.gpsimd if ky == 2 else None)
                        hs.append(h)
                    g0t = rpool.tile([128, O * 40], F16, tag="g0")
                    nc.vector.tensor_tensor(
                        out=g0t[:tw, :], in0=hs[0][:tw, :],
                        in1=hs[1][:tw, :], op=mybir.AluOpType.add)
                    g1t = rpool.tile([128, O * 40], F16, tag="g1")
                    nc.vector.tensor_tensor(
                        out=g1t[:tw, :], in0=g0t[:tw, :],
                        in1=hs[2][:tw, :], op=mybir.AluOpType.add)

                    l20 = rpool.tile([128, O * 20], F16, tag="l20")
                    fold(g1t, 40 * O, l20)
                    l10 = rpool.tile([128, O * 10], F16, tag="l10")
                    fold(l20, 20 * O, l10)
                    l5 = rpool.tile([128, O * 5], F16, tag="l5")
                    fold(l10, 10 * O, l5)
                    # 5 remaining 72-wide terms: 4+1 finish
                    f1 = rpool.tile([128, 2 * O], F16, tag="f1")
                    fold(l5, 4 * O, f1)
                    f2 = rpool.tile([128, O], F16, tag="f2")
                    fold(f1, 2 * O, f2)
                    acc = rpool.tile([128, O], F32, tag="acc")
                    nc.vector.tensor_tensor(
                        out=acc[:tw, :], in0=f2[:tw, :],
                        in1=l5[:tw, 4 * O:5 * O], op=mybir.AluOpType.add)

                    orow = r * W + (c0 - PADC)
                    nc.sync.dma_start(out_d[orow:orow + tw, :], acc[:tw, :])

    nc.compile()
    return nc


# ------------------------- host side -------------------------

_nc_cache = [None]


def _get_nc():
    if _nc_cache[0] is None:
        _nc_cache[0] = build_module()
    return _nc_cache[0]


def _consts(weight, off_w, off_b):
    # wk[k, c, o]: block-diag group conv weights for tap k
    wk = np.zeros((NK, C, O), np.float32)
    for g in range(9):
        for og in range(8):
            for cg in range(8):
                for k in range(NK):
                    wk[k, g * 8 + cg, g * 8 + og] = weight[
                        g * 8 + og, cg, k // 3, k % 3]
    # wts columns: slot-major [slot(48) x O]; slot = ky*16 + i (SLOT15 order)
    wts = np.zeros((C, NSLOT * O), np.float16)
    for ky in range(3):
        for i, (s, kx, v) in enumerate(SLOT15):
            k = ky * 3 + kx
            j = ky * NSPK + i
            wts[:, j * O:(j + 1) * O] = wk[k].astype(np.float16)

    offw = np.zeros((C, 9 * OC), np.float16)
    for t in range(9):
        offw[:, t * OC:(t + 1) * OC] = off_w[:, :, t // 3, t % 3].T

    # q rows: row = ky*80 + u*16 + i ; q = tent(dy_k-(u-2)) * tent(dx_k-(v-2))
    repy = np.zeros((OC, NQROW), np.float16)
    repx = np.zeros((OC, NQROW), np.float16)
    biasu = np.full((NQROW, 1), -3.0, np.float32)   # pads stay 0 via tent(-3)
    biasv = np.zeros((NQROW, 1), np.float32)
    for ky in range(3):
        for u in range(5):
            for i, (s, kx, v) in enumerate(SLOT15):
                k = ky * 3 + kx
                row = ky * NTPK + u * NSPK + i
                repy[2 * k, row] = 1.0
                repx[2 * k + 1, row] = 1.0
                biasu[row] = -(u - 2)
                biasv[row] = -(v - 2)
    return {
        "wts": wts, "offw": offw,
        "offb": off_b.reshape(OC, 1).astype(np.float32),
        "repy": repy, "repx": repx, "biasu": biasu, "biasv": biasv,
        "ident": np.eye(128, dtype=np.float32),
    }


def _slab(x_b, halo, rows):
    out = []
    for q in range(NQ):
        s = np.zeros((C, rows, WP), np.float16)
        lo, hi = q * RS - halo, q * RS + RS + halo
        clo, chi = max(lo, 0), min(hi, H)
        s[:, clo - lo: clo - lo + (chi - clo), PADC:PADC + W] = x_b[:, clo:chi]
        out.append(np.ascontiguousarray(s.reshape(C, rows * WP)))
    return out


def kernel(input, offset_feat, weight, off_w, off_b):
    input = np.asarray(input, np.float32)
    offset_feat = np.asarray(offset_feat, np.float32)
    weight = np.asarray(weight, np.float32)
    off_w = np.asarray(off_w, np.float32)
    off_b = np.asarray(off_b, np.float32)

    nc = _get_nc()
    consts = _consts(weight, off_w, off_b)
    in_maps = []
    for b in range(B):
        imgs = _slab(input[b], HALO, RSP)
        feats = _slab(offset_feat[b], 1, FROWS)
        for q in range(NQ):
            m = dict(consts)
            m["img"] = imgs[q]
            m["feat"] = feats[q]
            in_maps.append(m)

    res = bass_utils.run_bass_kernel_spmd(
        nc, in_maps, core_ids=list(range(N_CORES)))

    out = np.empty((B, O, H, W), np.float32)
    for ci in range(N_CORES):
        b, q = ci // NQ, ci % NQ
        o = res.results[ci]["out"]
        out[b, :, q * RS:(q + 1) * RS, :] = (
            o.reshape(RS, W, O).transpose(2, 0, 1))
    return out


if __name__ == "__main__":
    import reference as ref
    inputs = {k: np.asarray(v) for k, v in ref.setup_inputs().items()}
    got = kernel(**inputs)
    print("out", got.shape, got.dtype)
